# revision 16
# baseline (speedup 1.0000x reference)
"""Trainium2 Bass kernel: per-channel 8x8 box-sum pooling, stride 4 (NCHW).

Input  x: (8, 32, 512, 512) f32  ->  output (8, 32, 127, 127) f32.

Sharding: data-parallel over the batch dim — image b runs on NeuronCore b
(zero communication).

Per core (32 channel planes of 512 x 512):

  1. Input DMA, one plane per dma_start: SBUF layout [128, e*512] with
     partition p holding input rows 4p..4p+3 — each partition's span is
     8 KiB of *contiguous* DRAM, so descriptors are 8 KiB and each 1-MiB
     plane stripes across all 16 DMA engines (64-KiB chunk round-robin).
  2. Vertical pooling on the tensor engine in f32r: with 4 consecutive rows
     per partition, window i covers exactly partitions i and i+1, for every
     row-chunk e. So all 4 accumulating matmuls share one bidiagonal weight
     matrix M[p, i] = (p==i) | (p==i+1):  V[i, w] = sum_e (M.T @ X_e)[i, w]
     = sum_{dh<8} x[4i+dh, w].  f32r rounds the data operand's mantissa
     (weights are exact 0/1); rel err ~1e-4 vs the 2e-2 gate.
  3. Horizontal pooling on the vector engine, reading PSUM [128, 512]:
     copy evens to SBUF, a[u]=V[2u]+V[2u+1] (one PSUM operand max per DVE
     op); b[m]=a[2m]+a[2m+1]; out[i,j]=b[j]+b[j+1].
  4. The final add writes into an SBUF staging tile (row i of plane c at
     free offset k*128 for the k-th plane of the group). Group stores of
     16/14/2 planes write DRAM out[i, c, j] — i-major, so each partition's
     group span is contiguous -> 8-KiB-scale descriptors striped across all
     DMA engines (instead of 32 per-plane 65-KiB stores that all land on
     DMA engine 0), and the 1-MiB group-0 store overlaps the input stream
     while the last store is only 128 KiB of tail. The host transposes
     [128, 32, 128] -> [32, 127, 127].

Row 127 of V (weight column 127 is all-zero) and output column 127 carry
zeros/garbage; the host slices both pads off.
"""

import numpy as np

B, C, H, W = 8, 32, 512, 512
KS, ST = 8, 4
HO = (H - KS) // ST + 1  # 127
WO = (W - KS) // ST + 1  # 127
P = 128
E = H // P  # 4 rows per partition
GROUPS = ((0, 16), (16, 30), (30, 32))  # output store groups [a, b)

_CACHE: dict = {}


def _pool_matrix() -> np.ndarray:
    # M[p, i] = 1 iff output row i's 8-row window covers partition p's rows
    # (4p..4p+3), i.e. p == i or p == i+1. Column 127 is zero padding.
    m = np.zeros((P, P), dtype=np.float32)
    i = np.arange(HO)
    m[i, i] = 1.0
    m[i + 1, i] = 1.0
    return m


def _build(repeat: int = 1):
    import concourse.bacc as bacc
    import concourse.mybir as mybir
    import concourse.tile as tile

    f32 = mybir.dt.float32
    f32r = mybir.dt.float32r

    nc = bacc.Bacc("TRN2", target_bir_lowering=False, debug=False, num_devices=B)
    x_t = nc.dram_tensor("x", [C, H, W], f32r, kind="ExternalInput")
    mv_t = nc.dram_tensor("mv", [P, P], f32r, kind="ExternalInput")
    # out[i, c, j]: i-major so each partition's store span is contiguous DRAM
    out_t = nc.dram_tensor("out", [P, C, P], f32, kind="ExternalOutput")

    # [c, p, (e w)]: partition p holds rows 4p..4p+3 of plane c
    x_ap = x_t.ap().rearrange("c (p e) w -> c p (e w)", p=P)
    out_ap = out_t.ap()

    with tile.TileContext(nc) as tc:
        with (
            tc.tile_pool(name="xin", bufs=8) as xin,
            tc.tile_pool(name="vpsum", bufs=8, space="PSUM") as vpsum,
            tc.tile_pool(name="tmp", bufs=6) as tmp,
            tc.tile_pool(name="stage", bufs=len(GROUPS) + 1) as stage,
        ):
            mv = stage.tile([P, P], f32r)
            nc.sync.dma_start(mv, mv_t.ap())
            for _ in range(repeat):
                st = None
                gi = 0
                for c in range(C):
                    ga, gb = GROUPS[gi]
                    xt = xin.tile([P, E * W], f32r)
                    nc.sync.dma_start(xt, x_ap[c])
                    if c == ga:
                        st = stage.tile([P, (gb - ga) * P], f32)
                    v = vpsum.tile([P, W], f32)
                    for e in range(E):
                        nc.tensor.matmul(
                            v,
                            mv,
                            xt[:, e * W : (e + 1) * W],
                            start=(e == 0),
                            stop=(e == E - 1),
                        )
                    v2 = v[:].rearrange("i (u two) -> i u two", two=2)
                    a0 = tmp.tile([P, W // 2], f32)
                    nc.vector.tensor_copy(a0, v2[:, :, 0])
                    a = tmp.tile([P, W // 2], f32)
                    nc.vector.tensor_add(a, v2[:, :, 1], a0)
                    a2 = a[:].rearrange("i (m two) -> i m two", two=2)
                    b = tmp.tile([P, W // 4], f32)
                    nc.vector.tensor_add(b, a2[:, :, 0], a2[:, :, 1])
                    o = (c - ga) * P
                    nc.vector.tensor_add(
                        st[:, o : o + WO], b[:, 0:WO], b[:, 1 : WO + 1]
                    )
                    if c + 1 == gb:
                        # pad column WO of each plane is stored but never
                        # read by the host; it stays unwritten SBUF
                        nc.scalar.dma_start(
                            out_ap[:, ga:gb, :],
                            st[:].rearrange("i (c j) -> i c j", j=P),
                        )
                        gi = (gi + 1) % len(GROUPS)
    nc.compile()
    return nc


def kernel(x: np.ndarray) -> np.ndarray:
    from concourse import bass_utils

    nc = _CACHE.get("nc")
    if nc is None:
        nc = _CACHE["nc"] = _build()
    x = np.ascontiguousarray(np.asarray(x, dtype=np.float32))
    assert x.shape == (B, C, H, W)
    mv = _pool_matrix()
    in_maps = [{"x": x[b], "mv": mv} for b in range(B)]
    res = bass_utils.run_bass_kernel_spmd(nc, in_maps, core_ids=list(range(B)))
    # out[i, c, j] -> [c, i, j], drop the i/j pads
    return np.stack(
        [res.results[b]["out"].transpose(1, 0, 2)[:, :HO, :WO] for b in range(B)],
        axis=0,
    )


# revision 21
# speedup vs baseline: 1.0080x; 1.0080x over previous
"""Trainium2 Bass kernel: per-channel 8x8 box-sum pooling, stride 4 (NCHW).

Input  x: (8, 32, 512, 512) f32  ->  output (8, 32, 127, 127) f32.

Sharding: data-parallel over the batch dim — image b runs on NeuronCore b
(zero communication).

Per core (32 channel planes of 512 x 512):

  1. Input DMA, one plane per dma_start: SBUF layout [128, e*512] with
     partition p holding input rows 4p..4p+3 — each partition's span is
     8 KiB of *contiguous* DRAM, so descriptors are 8 KiB and each 1-MiB
     plane stripes across all 16 DMA engines (64-KiB chunk round-robin).
  2. Vertical pooling on the tensor engine in f32r: with 4 consecutive rows
     per partition, window i covers exactly partitions i and i+1, for every
     row-chunk e. So all 4 accumulating matmuls share one bidiagonal weight
     matrix M[p, i] = (p==i) | (p==i+1):  V[i, w] = sum_e (M.T @ X_e)[i, w]
     = sum_{dh<8} x[4i+dh, w].  f32r rounds the data operand's mantissa
     (weights are exact 0/1); rel err ~1e-4 vs the 2e-2 gate. M is built
     on-device (memset + two affine_selects + f32r-rounding copy): a DMA'd
     64-KiB weight load would ride entirely on DMA engine 0 (every
     instruction's first 64-KiB chunk goes to engine 0) and make it the
     input-stream straggler.
  3. Horizontal pooling on the vector engine, reading PSUM [128, 512]:
     copy evens to SBUF, a[u]=V[2u]+V[2u+1] (one PSUM operand max per DVE
     op); b[m]=a[2m]+a[2m+1]; out[i,j]=b[j]+b[j+1].
  4. The final add writes into an SBUF staging tile (row i of plane c at
     free offset k*128 for the k-th plane of the group). Group stores of
     16/14/2 planes write DRAM out[i, c, j] — i-major, so each partition's
     group span is contiguous -> 8-KiB-scale descriptors striped across all
     DMA engines (instead of 32 per-plane 65-KiB stores that all land on
     DMA engine 0), and the 1-MiB group-0 store overlaps the input stream
     while the last store is only 128 KiB of tail. The host transposes
     [128, 32, 128] -> [32, 127, 127].

Row 127 of V (weight column 127 is all-zero) and output column 127 carry
zeros/garbage; the host slices both pads off.
"""

import numpy as np

B, C, H, W = 8, 32, 512, 512
KS, ST = 8, 4
HO = (H - KS) // ST + 1  # 127
WO = (W - KS) // ST + 1  # 127
P = 128
E = H // P  # 4 rows per partition
GROUPS = ((0, 16), (16, 30), (30, 32))  # output store groups [a, b)

_CACHE: dict = {}


def _pool_matrix() -> np.ndarray:
    # M[p, i] = 1 iff output row i's 8-row window covers partition p's rows
    # (4p..4p+3), i.e. p == i or p == i+1. Column 127 is zero padding.
    m = np.zeros((P, P), dtype=np.float32)
    i = np.arange(HO)
    m[i, i] = 1.0
    m[i + 1, i] = 1.0
    return m


def _build(repeat: int = 1):
    import concourse.bacc as bacc
    import concourse.mybir as mybir
    import concourse.tile as tile

    f32 = mybir.dt.float32
    f32r = mybir.dt.float32r

    nc = bacc.Bacc("TRN2", target_bir_lowering=False, debug=False, num_devices=B)
    x_t = nc.dram_tensor("x", [C, H, W], f32r, kind="ExternalInput")
    # out[i, c, j]: i-major so each partition's store span is contiguous DRAM
    out_t = nc.dram_tensor("out", [P, C, P], f32, kind="ExternalOutput")

    # [c, p, (e w)]: partition p holds rows 4p..4p+3 of plane c
    x_ap = x_t.ap().rearrange("c (p e) w -> c p (e w)", p=P)
    out_ap = out_t.ap()

    with tile.TileContext(nc) as tc:
        with (
            tc.tile_pool(name="xin", bufs=8) as xin,
            tc.tile_pool(name="vpsum", bufs=8, space="PSUM") as vpsum,
            tc.tile_pool(name="tmp", bufs=6) as tmp,
            tc.tile_pool(name="stage", bufs=len(GROUPS) + 1) as stage,
        ):
            # build M[p, i] = (0 <= p - i <= 1) on-device: a DMA'd 64-KiB
            # weight load would ride entirely on DMA engine 0 (every
            # instruction's first 64-KiB chunk goes to engine 0), making it
            # the stream straggler
            mvf = stage.tile([P, P], f32)
            nc.gpsimd.memset(mvf, 1.0)
            nc.gpsimd.affine_select(
                out=mvf[:], in_=mvf[:], compare_op=mybir.AluOpType.is_ge,
                fill=0.0, base=0, pattern=[[-1, P]], channel_multiplier=1,
            )
            nc.gpsimd.affine_select(
                out=mvf[:], in_=mvf[:], compare_op=mybir.AluOpType.is_ge,
                fill=0.0, base=1, pattern=[[1, P]], channel_multiplier=-1,
            )
            mv = stage.tile([P, P], f32r)
            nc.vector.tensor_copy(mv, mvf[:])
            for _ in range(repeat):
                st = None
                gi = 0
                for c in range(C):
                    ga, gb = GROUPS[gi]
                    xt = xin.tile([P, E * W], f32r)
                    nc.sync.dma_start(xt, x_ap[c])
                    if c == ga:
                        st = stage.tile([P, (gb - ga) * P], f32)
                    v = vpsum.tile([P, W], f32)
                    for e in range(E):
                        nc.tensor.matmul(
                            v,
                            mv,
                            xt[:, e * W : (e + 1) * W],
                            start=(e == 0),
                            stop=(e == E - 1),
                        )
                    v2 = v[:].rearrange("i (u two) -> i u two", two=2)
                    a0 = tmp.tile([P, W // 2], f32)
                    nc.vector.tensor_copy(a0, v2[:, :, 0])
                    a = tmp.tile([P, W // 2], f32)
                    nc.vector.tensor_add(a, v2[:, :, 1], a0)
                    a2 = a[:].rearrange("i (m two) -> i m two", two=2)
                    b = tmp.tile([P, W // 4], f32)
                    nc.vector.tensor_add(b, a2[:, :, 0], a2[:, :, 1])
                    o = (c - ga) * P
                    nc.vector.tensor_add(
                        st[:, o : o + WO], b[:, 0:WO], b[:, 1 : WO + 1]
                    )
                    if c + 1 == gb:
                        # pad column WO of each plane is stored but never
                        # read by the host; it stays unwritten SBUF.
                        # Mid-stream stores issue from the ACT queue (the SP
                        # queue still holds pending input loads); the final
                        # store uses the by-then-idle SP queue (shorter DGE
                        # delay on the tail).
                        eng = nc.sync if c + 1 == C else nc.scalar
                        eng.dma_start(
                            out_ap[:, ga:gb, :],
                            st[:].rearrange("i (c j) -> i c j", j=P),
                        )
                        gi = (gi + 1) % len(GROUPS)
    nc.compile()
    return nc


def kernel(x: np.ndarray) -> np.ndarray:
    from concourse import bass_utils

    nc = _CACHE.get("nc")
    if nc is None:
        nc = _CACHE["nc"] = _build()
    x = np.ascontiguousarray(np.asarray(x, dtype=np.float32))
    assert x.shape == (B, C, H, W)
    in_maps = [{"x": x[b]} for b in range(B)]
    res = bass_utils.run_bass_kernel_spmd(nc, in_maps, core_ids=list(range(B)))
    # out[i, c, j] -> [c, i, j], drop the i/j pads
    return np.stack(
        [res.results[b]["out"].transpose(1, 0, 2)[:, :HO, :WO] for b in range(B)],
        axis=0,
    )


# revision 22
# speedup vs baseline: 1.0204x; 1.0123x over previous
"""Trainium2 Bass kernel: per-channel 8x8 box-sum pooling, stride 4 (NCHW).

Input  x: (8, 32, 512, 512) f32  ->  output (8, 32, 127, 127) f32.

Sharding: data-parallel over the batch dim — image b runs on NeuronCore b
(zero communication).

Per core (32 channel planes of 512 x 512):

  1. Input DMA, one plane per dma_start: SBUF layout [128, e*512] with
     partition p holding input rows 4p..4p+3 — each partition's span is
     8 KiB of *contiguous* DRAM, so descriptors are 8 KiB and each 1-MiB
     plane stripes across all 16 DMA engines (64-KiB chunk round-robin).
  2. Vertical pooling on the tensor engine in f32r: with 4 consecutive rows
     per partition, window i covers exactly partitions i and i+1, for every
     row-chunk e. So all 4 accumulating matmuls share one bidiagonal weight
     matrix M[p, i] = (p==i) | (p==i+1):  V[i, w] = sum_e (M.T @ X_e)[i, w]
     = sum_{dh<8} x[4i+dh, w].  f32r rounds the data operand's mantissa
     (weights are exact 0/1); rel err ~1e-4 vs the 2e-2 gate. M is built
     on-device (memset + two affine_selects + f32r-rounding copy): a DMA'd
     64-KiB weight load would ride entirely on DMA engine 0 (every
     instruction's first 64-KiB chunk goes to engine 0) and make it the
     input-stream straggler.
  3. Horizontal pooling on the vector engine, reading PSUM [128, 512]:
     one innermost-dim tensor_reduce gives non-overlapping 4-sums
     b[m] = sum V[4m..4m+3]; then out[i,j] = b[j] + b[j+1].
  4. The final add writes into an SBUF staging tile (row i of plane c at
     free offset k*128 for the k-th plane of the group). Group stores of
     16/14/2 planes write DRAM out[i, c, j] — i-major, so each partition's
     group span is contiguous -> 8-KiB-scale descriptors striped across all
     DMA engines (instead of 32 per-plane 65-KiB stores that all land on
     DMA engine 0), and the 1-MiB group-0 store overlaps the input stream
     while the last store is only 128 KiB of tail. The host transposes
     [128, 32, 128] -> [32, 127, 127].

Row 127 of V (weight column 127 is all-zero) and output column 127 carry
zeros/garbage; the host slices both pads off.
"""

import numpy as np

B, C, H, W = 8, 32, 512, 512
KS, ST = 8, 4
HO = (H - KS) // ST + 1  # 127
WO = (W - KS) // ST + 1  # 127
P = 128
E = H // P  # 4 rows per partition
GROUPS = ((0, 16), (16, 30), (30, 32))  # output store groups [a, b)

_CACHE: dict = {}


def _pool_matrix() -> np.ndarray:
    # M[p, i] = 1 iff output row i's 8-row window covers partition p's rows
    # (4p..4p+3), i.e. p == i or p == i+1. Column 127 is zero padding.
    m = np.zeros((P, P), dtype=np.float32)
    i = np.arange(HO)
    m[i, i] = 1.0
    m[i + 1, i] = 1.0
    return m


def _build(repeat: int = 1):
    import concourse.bacc as bacc
    import concourse.mybir as mybir
    import concourse.tile as tile

    f32 = mybir.dt.float32
    f32r = mybir.dt.float32r

    nc = bacc.Bacc("TRN2", target_bir_lowering=False, debug=False, num_devices=B)
    x_t = nc.dram_tensor("x", [C, H, W], f32r, kind="ExternalInput")
    # out[i, c, j]: i-major so each partition's store span is contiguous DRAM
    out_t = nc.dram_tensor("out", [P, C, P], f32, kind="ExternalOutput")

    # [c, p, (e w)]: partition p holds rows 4p..4p+3 of plane c
    x_ap = x_t.ap().rearrange("c (p e) w -> c p (e w)", p=P)
    out_ap = out_t.ap()

    with tile.TileContext(nc) as tc:
        with (
            tc.tile_pool(name="xin", bufs=8) as xin,
            tc.tile_pool(name="vpsum", bufs=8, space="PSUM") as vpsum,
            tc.tile_pool(name="tmp", bufs=6) as tmp,
            tc.tile_pool(name="stage", bufs=len(GROUPS) + 1) as stage,
        ):
            # build M[p, i] = (0 <= p - i <= 1) on-device: a DMA'd 64-KiB
            # weight load would ride entirely on DMA engine 0 (every
            # instruction's first 64-KiB chunk goes to engine 0), making it
            # the stream straggler
            mvf = stage.tile([P, P], f32)
            nc.gpsimd.memset(mvf, 1.0)
            nc.gpsimd.affine_select(
                out=mvf[:], in_=mvf[:], compare_op=mybir.AluOpType.is_ge,
                fill=0.0, base=0, pattern=[[-1, P]], channel_multiplier=1,
            )
            nc.gpsimd.affine_select(
                out=mvf[:], in_=mvf[:], compare_op=mybir.AluOpType.is_ge,
                fill=0.0, base=1, pattern=[[1, P]], channel_multiplier=-1,
            )
            mv = stage.tile([P, P], f32r)
            nc.vector.tensor_copy(mv, mvf[:])
            for _ in range(repeat):
                st = None
                gi = 0
                for c in range(C):
                    ga, gb = GROUPS[gi]
                    xt = xin.tile([P, E * W], f32r)
                    nc.sync.dma_start(xt, x_ap[c])
                    if c == ga:
                        st = stage.tile([P, (gb - ga) * P], f32)
                    v = vpsum.tile([P, W], f32)
                    for e in range(E):
                        nc.tensor.matmul(
                            v,
                            mv,
                            xt[:, e * W : (e + 1) * W],
                            start=(e == 0),
                            stop=(e == E - 1),
                        )
                    # non-overlapping 4-sums in ONE DVE op (innermost-dim
                    # reduce; single PSUM operand), then the stride-4 8-wide
                    # window is b[j] + b[j+1]
                    v4 = v[:].rearrange("i (m four) -> i m four", four=4)
                    b = tmp.tile([P, W // 4], f32)
                    nc.vector.tensor_reduce(
                        b, v4, axis=mybir.AxisListType.X, op=mybir.AluOpType.add
                    )
                    o = (c - ga) * P
                    nc.vector.tensor_add(
                        st[:, o : o + WO], b[:, 0:WO], b[:, 1 : WO + 1]
                    )
                    if c + 1 == gb:
                        # pad column WO of each plane is stored but never
                        # read by the host; it stays unwritten SBUF.
                        # Mid-stream stores issue from the ACT queue (the SP
                        # queue still holds pending input loads); the final
                        # store uses the by-then-idle SP queue (shorter DGE
                        # delay on the tail).
                        eng = nc.sync if c + 1 == C else nc.scalar
                        eng.dma_start(
                            out_ap[:, ga:gb, :],
                            st[:].rearrange("i (c j) -> i c j", j=P),
                        )
                        gi = (gi + 1) % len(GROUPS)
    nc.compile()
    return nc


def kernel(x: np.ndarray) -> np.ndarray:
    from concourse import bass_utils

    nc = _CACHE.get("nc")
    if nc is None:
        nc = _CACHE["nc"] = _build()
    x = np.ascontiguousarray(np.asarray(x, dtype=np.float32))
    assert x.shape == (B, C, H, W)
    in_maps = [{"x": x[b]} for b in range(B)]
    res = bass_utils.run_bass_kernel_spmd(nc, in_maps, core_ids=list(range(B)))
    # out[i, c, j] -> [c, i, j], drop the i/j pads
    return np.stack(
        [res.results[b]["out"].transpose(1, 0, 2)[:, :HO, :WO] for b in range(B)],
        axis=0,
    )
